# revision 13
# baseline (speedup 1.0000x reference)
"""Trainium2 Bass kernel for the BronxLayer GNN message-passing problem.

Strategy (8 NeuronCores, SPMD, no collectives):
  - Shard the query-node axis x (first N of the [N,N,heads] tensors) across
    cores: 512 query rows per core.  Weights + h are replicated; each core
    computes the full LayerNorm hn locally.
  - On-chip layout is "L2": key-node axis y on SBUF partitions, query axis x
    on the free dimension.  The sampled attention tensor `a` is then born
    transposed, exactly the layout the message-passing matmul needs as its
    moving operand (contraction over y on partitions) -- no transposes of the
    big [N,N] tensors anywhere on device.
  - eps / diffusion are pre-transposed + bf16-cast on the host (sharding /
    layout prep), making every DMA a clean contiguous transfer.
  - softplus is composed as Ln(Exp(ls)+1) (both in the single
    natural_log_exp table set -> no ACT table switches; LayerNorm rstd uses
    exp(-0.5*ln(var+eps)) for the same reason).
  - t = mu + sigma*eps is built in PSUM by letting the TensorEngine add
    p = sigma*eps into the score accumulation (identity matmul, start=True,
    then the mu score matmul with start=False) -- saves a full PSUM-sourced
    DVE pass over the 34M-element score tensor.
  - Row sums (the L1 denominators) come from ones-column matmuls
    accumulating in PSUM; KL sums ride on ACT accum_out / DVE
    scalar_tensor_tensor accum_out; sum(mu_s^2) uses the Gram identity
    ||K M^T||_F^2 = tr((K^T K)(M^T M)).
"""

import sys

for _p in ("/opt/trn_rl_repo", "/root/.axon_site/_ro/trn_rl_repo"):
    if _p not in sys.path:
        sys.path.insert(0, _p)

import numpy as np

import concourse.bass as bass
import concourse.bacc as bacc
import concourse.tile as tile
from concourse import mybir
from concourse.masks import make_identity

BF16 = mybir.dt.bfloat16
F32 = mybir.dt.float32
NP_BF16 = mybir.dt.np(BF16)
ALU = mybir.AluOpType
ACT = mybir.ActivationFunctionType
AX = mybir.AxisListType

N_CORES = 8
LN_EPS = 1e-5
L1_EPS = 1e-12


def build(n=4096, xs=512, hdim=128, heads=2, chunk_blks=8):
    """Build the per-core SPMD Bass program.  Returns nc."""
    dh = hdim // heads
    nblk = n // 128
    nchunk = nblk // chunk_blks
    xtiles = xs // 128
    nacc = nchunk * heads  # accum slots per quantity
    assert xs <= 512 and xs % 128 == 0 and n % (128 * chunk_blks) == 0

    nc = bacc.Bacc()

    h_d = nc.declare_dram_parameter("h", [n, hdim], F32, isOutput=False)
    hx_d = nc.declare_dram_parameter("h_x", [xs, hdim], F32, isOutput=False)
    gamma_d = nc.declare_dram_parameter("gamma", [1, hdim], F32, isOutput=False)
    beta_d = nc.declare_dram_parameter("beta", [1, hdim], F32, isOutput=False)
    bv_d = nc.declare_dram_parameter("b_v", [hdim, 1], F32, isOutput=False)
    wk_d = nc.declare_dram_parameter("wk2T", [hdim, hdim], BF16, isOutput=False)
    wmu_d = nc.declare_dram_parameter("wmu2T", [hdim, hdim], BF16, isOutput=False)
    wls_d = nc.declare_dram_parameter("wls2T", [hdim, hdim], BF16, isOutput=False)
    wv0_d = nc.declare_dram_parameter("wv0T", [hdim, hdim], BF16, isOutput=False)
    wv1_d = nc.declare_dram_parameter("wv1T", [hdim, hdim], BF16, isOutput=False)
    eps_d = nc.declare_dram_parameter("epsT", [heads, n, xs], BF16, isOutput=False)
    diff_d = nc.declare_dram_parameter("diffT", [n, xs], BF16, isOutput=False)
    ot_d = nc.declare_dram_parameter("o_t", [hdim, xs], F32, isOutput=True)
    kl_d = nc.declare_dram_parameter("kl", [1, 1], F32, isOutput=True)

    def bcast_row(ap2d, parts=128):
        # [1, d] DRAM row -> broadcast AP over `parts` partitions
        return bass.AP(tensor=ap2d.tensor, offset=ap2d.offset,
                       ap=[[0, parts]] + [list(ap2d.ap[-1])])

    from contextlib import ExitStack
    with tile.TileContext(nc) as tc, ExitStack() as stack:
        consts = stack.enter_context(tc.tile_pool(name="consts", bufs=1))
        big = stack.enter_context(tc.tile_pool(name="big", bufs=1))
        msgp = stack.enter_context(tc.tile_pool(name="msgp", bufs=1, space="PSUM"))

        # ---------------- constants ----------------
        id_bf = consts.tile([128, 128], BF16, tag="id_bf")
        make_identity(nc, id_bf)
        ones_bf = consts.tile([128, 1], BF16, tag="ones_bf")
        nc.vector.memset(ones_bf, 1.0)
        ones1f = consts.tile([1, 128], F32, tag="ones1f")
        nc.vector.memset(ones1f, 1.0)
        mask3 = consts.tile([128, 3], F32, tag="mask3")
        nc.vector.memset(mask3, 1.0)
        nc.vector.memset(mask3[dh:128, 1:2], 0.0)
        nc.vector.memset(mask3[0:dh, 2:3], 0.0)
        lneps_c = consts.tile([128, 1], F32, tag="lneps")
        nc.vector.memset(lneps_c, LN_EPS)
        gammaB = consts.tile([128, hdim], F32, tag="gammaB")
        nc.gpsimd.dma_start(out=gammaB, in_=bcast_row(gamma_d[:, :]))
        betaB = consts.tile([128, hdim], F32, tag="betaB")
        nc.gpsimd.dma_start(out=betaB, in_=bcast_row(beta_d[:, :]))
        bv_sb = consts.tile([hdim, 1], F32, tag="bv")
        nc.gpsimd.dma_start(out=bv_sb, in_=bv_d[:, :])
        wk_sb = consts.tile([128, 128], BF16, tag="wk")
        nc.gpsimd.dma_start(out=wk_sb, in_=wk_d[:, :])
        wmu_sb = consts.tile([128, 128], BF16, tag="wmu")
        nc.gpsimd.dma_start(out=wmu_sb, in_=wmu_d[:, :])
        wls_sb = consts.tile([128, 128], BF16, tag="wls")
        nc.gpsimd.dma_start(out=wls_sb, in_=wls_d[:, :])
        wv0_sb = consts.tile([128, 128], BF16, tag="wv0")
        nc.gpsimd.dma_start(out=wv0_sb, in_=wv0_d[:, :])
        wv1_sb = consts.tile([128, 128], BF16, tag="wv1")
        nc.gpsimd.dma_start(out=wv1_sb, in_=wv1_d[:, :])

        # ---------------- persistent big buffers ----------------
        hn_sb = big.tile([128, nblk, 128], BF16, tag="hn")        # hn, y-block tiles
        hnT_sb = big.tile([128, n], BF16, tag="hnT")              # hn^T [f, node]
        hnxT_sb = big.tile([128, xs], BF16, tag="hnxT")           # hn_x^T [f, xq]
        kT_sb = big.tile([128, xs], BF16, tag="kT")               # packed k^T
        muT_sb = big.tile([128, n], BF16, tag="muT")
        lsT_sb = big.tile([128, n], BF16, tag="lsT")
        kl_slab = big.tile([128, 2 * nacc + 2], F32, tag="klslab")
        scr = big.tile([128, chunk_blks * xs], BF16, tag="scr")   # lg/s2 scratch

        # persistent PSUM: message accumulators + denominators
        msg_ps = [msgp.tile([128, xs], F32, tag=f"msg{b}", name=f"msg{b}")
                  for b in range(heads)]
        den_ps = msgp.tile([64, xs], F32, tag="den")

        # ---------------- Phase 0: LayerNorm (+ transposes) ----------------
        with tc.tile_pool(name="lnp", bufs=3) as lnp, \
             tc.tile_pool(name="lnps", bufs=3, space="PSUM") as lnps:

            def ln_tile(src_slice, dst_bf_slice):
                x_t = lnp.tile([128, hdim], F32, tag="x")
                nc.gpsimd.dma_start(out=x_t, in_=src_slice)
                stats = lnp.tile([128, 6], F32, tag="stats")
                nc.vector.bn_stats(out=stats, in_=x_t)
                mv = lnp.tile([128, 2], F32, tag="mv")
                nc.vector.bn_aggr(out=mv, in_=stats)
                # rstd = exp(-0.5*ln(var+eps))  (stays in the ln/exp table set)
                u = lnp.tile([128, 1], F32, tag="u")
                nc.scalar.activation(out=u, in_=mv[:, 1:2], func=ACT.Ln,
                                     bias=lneps_c)
                r = lnp.tile([128, 1], F32, tag="r")
                nc.scalar.activation(out=r, in_=u, func=ACT.Exp, scale=-0.5)
                t1 = lnp.tile([128, hdim], F32, tag="t1")
                nc.vector.tensor_scalar(t1, x_t, mv[:, 0:1], r, ALU.subtract, ALU.mult)
                t2 = lnp.tile([128, hdim], F32, tag="t2")
                nc.vector.tensor_tensor(t2, t1, gammaB, ALU.mult)
                nc.vector.tensor_tensor(dst_bf_slice, t2, betaB, ALU.add)

            for blk in range(nblk):
                ln_tile(h_d[blk * 128:(blk + 1) * 128, :], hn_sb[:, blk, :])
                tp = lnps.tile([128, 128], BF16, tag="tp")
                nc.tensor.transpose(tp, hn_sb[:, blk, :], id_bf)
                nc.vector.tensor_copy(hnT_sb[:, blk * 128:(blk + 1) * 128], tp)
            for i in range(xtiles):
                hx_bf = lnp.tile([128, hdim], BF16, tag=f"hxbf{i}")
                ln_tile(hx_d[i * 128:(i + 1) * 128, :], hx_bf)
                tp = lnps.tile([128, 128], BF16, tag="tp")
                nc.tensor.transpose(tp, hx_bf, id_bf)
                nc.vector.tensor_copy(hnxT_sb[:, i * 128:(i + 1) * 128], tp)

        # ---------------- Phase 1: projections + Gram (sum mu^2) ----------
        with tc.tile_pool(name="prj", bufs=3) as prj, \
             tc.tile_pool(name="prjps", bufs=2, space="PSUM") as prjps, \
             tc.tile_pool(name="gramps", bufs=1, space="PSUM") as gramps:
            ps = prjps.tile([128, xs], F32, tag="pp")
            nc.tensor.matmul(ps, wk_sb, hnxT_sb, start=True, stop=True)
            nc.vector.tensor_copy(kT_sb, ps)
            for j in range(n // 512):
                sl = slice(j * 512, (j + 1) * 512)
                ps = prjps.tile([128, 512], F32, tag="pp")
                nc.tensor.matmul(ps, wmu_sb, hnT_sb[:, sl], start=True, stop=True)
                nc.vector.tensor_copy(muT_sb[:, sl], ps)
                ps = prjps.tile([128, 512], F32, tag="pp")
                nc.tensor.matmul(ps, wls_sb, hnT_sb[:, sl], start=True, stop=True)
                nc.vector.tensor_copy(lsT_sb[:, sl], ps)

            # Gram matrices: A = K^T K (over the x shard), B = M^T M (all nodes)
            A_ps = gramps.tile([128, 128], F32, tag="A")
            B_ps = gramps.tile([128, 128], F32, tag="B")
            for i in range(xtiles):
                ps = prjps.tile([128, 128], F32, tag="pp")
                nc.tensor.matmul(ps, hnxT_sb[:, i * 128:(i + 1) * 128], wk_sb,
                                 start=True, stop=True)
                kn = prj.tile([128, 128], BF16, tag="kn")
                nc.vector.tensor_copy(kn, ps)
                nc.tensor.matmul(A_ps, kn, kn, start=(i == 0), stop=(i == xtiles - 1))
            for j in range(nblk):
                ps = prjps.tile([128, 128], F32, tag="pp")
                nc.tensor.matmul(ps, hnT_sb[:, j * 128:(j + 1) * 128], wmu_sb,
                                 start=True, stop=True)
                mn = prj.tile([128, 128], BF16, tag="mn")
                nc.vector.tensor_copy(mn, ps)
                nc.tensor.matmul(B_ps, mn, mn, start=(j == 0), stop=(j == nblk - 1))
            A_sb = prj.tile([128, 128], F32, tag="Asb")
            nc.vector.tensor_copy(A_sb, A_ps)
            B_sb = prj.tile([128, 128], F32, tag="Bsb")
            nc.vector.tensor_copy(B_sb, B_ps)
            AB = prj.tile([128, 128], F32, tag="AB")
            nc.vector.tensor_tensor(AB, A_sb, B_sb, ALU.mult)
            nc.vector.tensor_reduce(kl_slab[:, 2 * nacc:2 * nacc + 1],
                                    AB[:, 0:dh], axis=AX.X, op=ALU.add)
            nc.vector.tensor_reduce(kl_slab[:, 2 * nacc + 1:2 * nacc + 2],
                                    AB[:, dh:128], axis=AX.X, op=ALU.add)

        # ---------------- Phase 2: main loop over key-node chunks ----------
        with tc.tile_pool(name="str", bufs=2) as strm, \
             tc.tile_pool(name="sigp", bufs=3) as sigp, \
             tc.tile_pool(name="mls", bufs=3, space="PSUM") as mls:
            for c in range(nchunk):
                y0 = c * chunk_blks * 128
                eps_t = []
                for b in range(heads):
                    et = strm.tile([128, chunk_blks, xs], BF16, tag=f"eps{b}")
                    src = eps_d[b, y0:y0 + chunk_blks * 128, :]
                    nc.sync.dma_start(
                        out=et, in_=src.rearrange("(blk p) x -> p blk x", p=128))
                    eps_t.append(et)
                dif_t = strm.tile([128, chunk_blks, xs], BF16, tag="dif")
                nc.sync.dma_start(
                    out=dif_t,
                    in_=diff_d[y0:y0 + chunk_blks * 128, :].rearrange(
                        "(blk p) x -> p blk x", p=128))

                ea_t = []
                for b in range(heads):
                    hs = slice(b * dh, (b + 1) * dh)
                    sig = sigp.tile([128, chunk_blks, xs], BF16, tag="sig")
                    # e1 = exp(ls) per block (drains score PSUM quickly)
                    for blk in range(chunk_blks):
                        yb = c * chunk_blks + blk
                        ls_ps = mls.tile([128, xs], F32, tag="ls")
                        nc.tensor.matmul(ls_ps,
                                         lsT_sb[hs, yb * 128:(yb + 1) * 128],
                                         kT_sb[hs, :], start=True, stop=True)
                        nc.scalar.activation(out=sig[:, blk, :], in_=ls_ps,
                                             func=ACT.Exp)
                    # sigma = ln(e1 + 1), chunked, in place
                    nc.scalar.activation(out=sig, in_=sig, func=ACT.Ln, bias=1.0)
                    # KL terms: sum ln(sigma), sum sigma^2
                    nc.scalar.activation(out=scr, in_=sig, func=ACT.Ln,
                                         accum_out=kl_slab[:, c * heads + b:
                                                           c * heads + b + 1])
                    nc.vector.scalar_tensor_tensor(
                        out=scr, in0=sig, scalar=1.0, in1=sig,
                        op0=ALU.bypass, op1=ALU.mult,
                        accum_out=kl_slab[:, nacc + c * heads + b:
                                          nacc + c * heads + b + 1])
                    # p = sigma * eps
                    p_t = sigp.tile([128, chunk_blks, xs], BF16, tag="p")
                    nc.vector.tensor_tensor(p_t, sig, eps_t[b], ALU.mult)
                    # t = p + mu (PE adds the mu scores on top of I@p), e = exp(t)
                    ea = sigp.tile([128, chunk_blks, xs], BF16, tag=f"ea{b}")
                    for blk in range(chunk_blks):
                        yb = c * chunk_blks + blk
                        t_ps = mls.tile([128, xs], F32, tag="t", bufs=2)
                        nc.tensor.matmul(t_ps, id_bf, p_t[:, blk, :],
                                         start=True, stop=False)
                        nc.tensor.matmul(t_ps,
                                         muT_sb[hs, yb * 128:(yb + 1) * 128],
                                         kT_sb[hs, :], start=False, stop=True)
                        nc.scalar.activation(out=ea[:, blk, :], in_=t_ps,
                                             func=ACT.Exp)
                    # a = e * diffusion (in place)
                    nc.vector.tensor_tensor(ea, ea, dif_t, ALU.mult)
                    ea_t.append(ea)

                # message passing + denominators
                for blk in range(chunk_blks):
                    yb = c * chunk_blks + blk
                    st = (yb == 0)
                    sp = (yb == nblk - 1)
                    for b in range(heads):
                        nc.tensor.matmul(msg_ps[b], hn_sb[:, yb, :],
                                         ea_t[b][:, blk, :], start=st, stop=sp)
                    nc.tensor.matmul(den_ps[0:1, :], ones_bf[:, 0:1],
                                     ea_t[0][:, blk, :], start=st, stop=sp,
                                     skip_group_check=True)
                    nc.tensor.matmul(den_ps[32:33, :], ones_bf[:, 0:1],
                                     ea_t[1][:, blk, :], start=st, stop=sp,
                                     skip_group_check=True)

        # ---------------- Phase 3: normalize, output proj, ELU, KL --------
        with tc.tile_pool(name="tl", bufs=1) as tl, \
             tc.tile_pool(name="tlps", bufs=1, space="PSUM") as tlps:
            msc = []
            for b in range(heads):
                d_sb = tl.tile([1, xs], F32, tag=f"d{b}")
                nc.vector.tensor_copy(d_sb, den_ps[32 * b:32 * b + 1, :])
                nc.vector.tensor_scalar_max(d_sb, d_sb, L1_EPS)
                nc.vector.reciprocal(d_sb, d_sb)
                rb_ps = tlps.tile([128, xs], F32, tag=f"rb{b}")
                nc.tensor.matmul(rb_ps, ones1f[0:1, :], d_sb, start=True, stop=True)
                rb_sb = tl.tile([128, xs], F32, tag=f"rbs{b}")
                nc.vector.tensor_copy(rb_sb, rb_ps)
                m_sb = tl.tile([128, xs], BF16, tag=f"msc{b}")
                nc.vector.tensor_tensor(m_sb, msg_ps[b], rb_sb, ALU.mult)
                msc.append(m_sb)
            o_ps = tlps.tile([128, xs], F32, tag="o")
            nc.tensor.matmul(o_ps, wv0_sb, msc[0], start=True, stop=False)
            nc.tensor.matmul(o_ps, wv1_sb, msc[1], start=False, stop=True)
            # elu(x) = max(x, min(0, exp(x) - 1)), with bias b_v folded in
            xb = tl.tile([128, xs], F32, tag="xb")
            nc.vector.tensor_scalar(xb, o_ps, bv_sb, None, ALU.add)
            ex = tl.tile([128, xs], F32, tag="ex")
            nc.scalar.activation(out=ex, in_=o_ps, func=ACT.Exp, bias=bv_sb)
            mn = tl.tile([128, xs], F32, tag="mn")
            nc.vector.tensor_scalar(mn, ex, -1.0, 0.0, ALU.add, ALU.min)
            ot = tl.tile([128, xs], F32, tag="ot")
            nc.vector.tensor_tensor(ot, xb, mn, ALU.max)
            nc.gpsimd.dma_start(out=ot_d[:, :], in_=ot)

            # KL assembly
            kw = 2 * nacc + 2
            klps = tlps.tile([1, kw], F32, tag="klps")
            nc.tensor.matmul(klps[0:1, 0:2 * nacc], mask3[:, 0:1],
                             kl_slab[:, 0:2 * nacc], start=True, stop=True)
            nc.tensor.matmul(klps[0:1, 2 * nacc:2 * nacc + 1], mask3[:, 1:2],
                             kl_slab[:, 2 * nacc:2 * nacc + 1],
                             start=True, stop=True)
            nc.tensor.matmul(klps[0:1, 2 * nacc + 1:kw], mask3[:, 2:3],
                             kl_slab[:, 2 * nacc + 1:kw], start=True, stop=True)
            ksb = tl.tile([1, kw], F32, tag="ksb")
            nc.vector.tensor_copy(ksb, klps)
            r_lg = tl.tile([1, 1], F32, tag="rlg")
            nc.vector.tensor_reduce(r_lg, ksb[0:1, 0:nacc], axis=AX.X, op=ALU.add)
            r_s2 = tl.tile([1, 1], F32, tag="rs2")
            nc.vector.tensor_reduce(r_s2, ksb[0:1, nacc:2 * nacc], axis=AX.X,
                                    op=ALU.add)
            mu2 = tl.tile([1, 1], F32, tag="mu2")
            nc.vector.tensor_tensor(mu2, ksb[0:1, 2 * nacc:2 * nacc + 1],
                                    ksb[0:1, 2 * nacc + 1:kw], ALU.add)
            u1 = tl.tile([1, 1], F32, tag="u1")
            nc.vector.tensor_scalar(u1, r_lg, -1.0, None, ALU.mult)
            u2 = tl.tile([1, 1], F32, tag="u2")
            nc.vector.tensor_scalar(u2, r_s2, 0.5, None, ALU.mult)
            nc.vector.tensor_tensor(u1, u1, u2, ALU.add)
            nc.vector.tensor_scalar(u2, mu2, 0.5, None, ALU.mult)
            nc.vector.tensor_tensor(u1, u1, u2, ALU.add)
            cnt = float(xs) * n * heads
            klv = tl.tile([1, 1], F32, tag="klv")
            nc.vector.tensor_scalar(klv, u1, 1.0 / (float(n) * n),
                                    -0.5 * cnt / (float(n) * n), ALU.mult, ALU.add)
            nc.gpsimd.dma_start(out=kl_d[:, :], in_=klv)

    nc.compile()
    return nc


# ------------------------------------------------------------------------
# Host side
# ------------------------------------------------------------------------

def prep_inputs(inputs, n=4096, xs=512, hdim=128, heads=2):
    """Shard + lay out the full-size numpy inputs into per-core in_maps."""
    h = np.asarray(inputs["h"], np.float32)
    diffusion = np.asarray(inputs["diffusion"], np.float32)
    eps = np.asarray(inputs["eps"], np.float32)
    w_k = np.asarray(inputs["w_k"], np.float32)
    w_mu = np.asarray(inputs["w_mu"], np.float32)
    w_ls = np.asarray(inputs["w_ls"], np.float32)
    w_v = np.asarray(inputs["w_v"], np.float32)
    b_v = np.asarray(inputs["b_v"], np.float32)
    gamma = np.asarray(inputs["gamma"], np.float32)
    beta = np.asarray(inputs["beta"], np.float32)

    scale = hdim ** -0.5

    def pack(w):
        return np.concatenate([w[b::heads] for b in range(heads)], 0)

    wk2T = np.ascontiguousarray((pack(w_k) * scale).T).astype(NP_BF16)
    wmu2T = np.ascontiguousarray(pack(w_mu).T).astype(NP_BF16)
    wls2T = np.ascontiguousarray(pack(w_ls).T).astype(NP_BF16)
    wv0T = np.ascontiguousarray(w_v[:, 0:hdim].T).astype(NP_BF16)
    wv1T = np.ascontiguousarray(w_v[:, hdim:2 * hdim].T).astype(NP_BF16)
    shared = {
        "h": np.ascontiguousarray(h),
        "gamma": np.ascontiguousarray(gamma.reshape(1, hdim)),
        "beta": np.ascontiguousarray(beta.reshape(1, hdim)),
        "b_v": np.ascontiguousarray(b_v.reshape(hdim, 1)),
        "wk2T": wk2T, "wmu2T": wmu2T, "wls2T": wls2T,
        "wv0T": wv0T, "wv1T": wv1T,
    }
    in_maps = []
    e0 = eps[0]                      # [N, N, heads]  (x, y, b)
    d0 = diffusion[:, :, 0]          # [N, N]         (x, y)
    for i in range(N_CORES):
        x0, x1 = i * xs, (i + 1) * xs
        m = dict(shared)
        m["h_x"] = np.ascontiguousarray(h[x0:x1])
        m["epsT"] = np.ascontiguousarray(
            e0[x0:x1].transpose(2, 1, 0)).astype(NP_BF16)
        m["diffT"] = np.ascontiguousarray(d0[x0:x1].T).astype(NP_BF16)
        in_maps.append(m)
    return in_maps


_NC_CACHE = {}


def _get_nc():
    if "nc" not in _NC_CACHE:
        _NC_CACHE["nc"] = build()
    return _NC_CACHE["nc"]


def _install_ntff_hook():
    """Shim antenv.axon_hooks (missing in this image) so trace=True works."""
    import types
    import antenv  # noqa: F401
    if "antenv.axon_hooks" in sys.modules:
        return
    mod = types.ModuleType("antenv.axon_hooks")
    mod._hook = None
    mod.set_axon_ntff_profile_hook = lambda h: setattr(mod, "_hook", h)
    mod.get_axon_ntff_profile_hook = lambda: mod._hook
    sys.modules["antenv.axon_hooks"] = mod
    setattr(antenv, "axon_hooks", mod)
    if "/root/.axon_site" not in sys.path:
        sys.path.insert(0, "/root/.axon_site")
    from trn_agent_boot.trn_boot import _ntff_profile_via_ctypes
    mod._hook = _ntff_profile_via_ctypes("/opt/axon/libaxon_pjrt.so")
    import concourse.bass_utils as bu
    bu.upload_artifacts = lambda d: d


def _run(inputs, trace=False):
    from concourse.bass_utils import run_bass_kernel_spmd
    if trace:
        _install_ntff_hook()
    nc = _get_nc()
    in_maps = prep_inputs(inputs)
    res = run_bass_kernel_spmd(nc, in_maps, list(range(N_CORES)), trace=trace)
    out = np.empty((4096, 128), np.float32)
    kl = np.float32(0.0)
    for i in range(N_CORES):
        out[i * 512:(i + 1) * 512, :] = res.results[i]["o_t"].T
        kl += np.float32(res.results[i]["kl"][0, 0])
    return (out, kl), res


def kernel(**inputs):
    (out, kl), _ = _run(inputs, trace=False)
    return out, kl


def kernel_timed(**inputs):
    (out, kl), res = _run(inputs, trace=True)
    return (out, kl), res.exec_time_ns


# revision 14
# speedup vs baseline: 1.3414x; 1.3414x over previous
"""Trainium2 Bass kernel for the BronxLayer GNN message-passing problem.

Strategy (8 NeuronCores, SPMD, no collectives):
  - Shard the query-node axis x (first N of the [N,N,heads] tensors) across
    cores: 512 query rows per core.  Weights + h are replicated; each core
    computes the full LayerNorm hn locally.
  - On-chip layout is "L2": key-node axis y on SBUF partitions, query axis x
    on the free dimension.  The sampled attention tensor `a` is then born
    transposed, exactly the layout the message-passing matmul needs as its
    moving operand (contraction over y on partitions) -- no transposes of the
    big [N,N] tensors anywhere on device.
  - eps / diffusion are pre-transposed + bf16-cast on the host (sharding /
    layout prep), making every DMA a clean contiguous transfer.
  - softplus is composed as Ln(Exp(ls)+1) (both in the single
    natural_log_exp table set -> no ACT table switches; LayerNorm rstd uses
    exp(-0.5*ln(var+eps)) for the same reason).
  - t = mu + sigma*eps is built in PSUM by letting the TensorEngine add
    p = sigma*eps into the score accumulation (identity matmul, start=True,
    then the mu score matmul with start=False) -- saves a full PSUM-sourced
    DVE pass over the 34M-element score tensor.
  - Row sums (the L1 denominators) come from ones-column matmuls
    accumulating in PSUM; KL sums ride on ACT accum_out / DVE
    scalar_tensor_tensor accum_out; sum(mu_s^2) uses the Gram identity
    ||K M^T||_F^2 = tr((K^T K)(M^T M)).
"""

import sys

for _p in ("/opt/trn_rl_repo", "/root/.axon_site/_ro/trn_rl_repo"):
    if _p not in sys.path:
        sys.path.insert(0, _p)

import numpy as np

import concourse.bass as bass
import concourse.bacc as bacc
import concourse.tile as tile
from concourse import mybir
from concourse.masks import make_identity

BF16 = mybir.dt.bfloat16
F32 = mybir.dt.float32
NP_BF16 = mybir.dt.np(BF16)
ALU = mybir.AluOpType
ACT = mybir.ActivationFunctionType
AX = mybir.AxisListType

N_CORES = 8
LN_EPS = 1e-5
L1_EPS = 1e-12


def build(n=4096, xs=512, hdim=128, heads=2, chunk_blks=8):
    """Build the per-core SPMD Bass program.  Returns nc."""
    dh = hdim // heads
    nblk = n // 128
    nchunk = nblk // chunk_blks
    xtiles = xs // 128
    nacc = nchunk * heads  # accum slots per quantity
    assert xs <= 512 and xs % 128 == 0 and n % (128 * chunk_blks) == 0

    nc = bacc.Bacc()

    h_d = nc.declare_dram_parameter("h", [n, hdim], F32, isOutput=False)
    hx_d = nc.declare_dram_parameter("h_x", [xs, hdim], F32, isOutput=False)
    gamma_d = nc.declare_dram_parameter("gamma", [1, hdim], F32, isOutput=False)
    beta_d = nc.declare_dram_parameter("beta", [1, hdim], F32, isOutput=False)
    bv_d = nc.declare_dram_parameter("b_v", [hdim, 1], F32, isOutput=False)
    wk_d = nc.declare_dram_parameter("wk2T", [hdim, hdim], BF16, isOutput=False)
    wmu_d = nc.declare_dram_parameter("wmu2T", [hdim, hdim], BF16, isOutput=False)
    wls_d = nc.declare_dram_parameter("wls2T", [hdim, hdim], BF16, isOutput=False)
    wv0_d = nc.declare_dram_parameter("wv0T", [hdim, hdim], BF16, isOutput=False)
    wv1_d = nc.declare_dram_parameter("wv1T", [hdim, hdim], BF16, isOutput=False)
    eps_d = nc.declare_dram_parameter("epsT", [heads, n, xs], BF16, isOutput=False)
    diff_d = nc.declare_dram_parameter("diffT", [n, xs], BF16, isOutput=False)
    ot_d = nc.declare_dram_parameter("o_t", [hdim, xs], F32, isOutput=True)
    kl_d = nc.declare_dram_parameter("kl", [1, 1], F32, isOutput=True)

    def bcast_row(ap2d, parts=128):
        # [1, d] DRAM row -> broadcast AP over `parts` partitions
        return bass.AP(tensor=ap2d.tensor, offset=ap2d.offset,
                       ap=[[0, parts]] + [list(ap2d.ap[-1])])

    from contextlib import ExitStack
    with tile.TileContext(nc) as tc, ExitStack() as stack:
        consts = stack.enter_context(tc.tile_pool(name="consts", bufs=1))
        big = stack.enter_context(tc.tile_pool(name="big", bufs=1))
        msgp = stack.enter_context(tc.tile_pool(name="msgp", bufs=1, space="PSUM"))

        # ---------------- constants ----------------
        id_bf = consts.tile([128, 128], BF16, tag="id_bf")
        make_identity(nc, id_bf)
        ones_bf = consts.tile([128, 1], BF16, tag="ones_bf")
        nc.vector.memset(ones_bf, 1.0)
        ones1f = consts.tile([1, 128], F32, tag="ones1f")
        nc.vector.memset(ones1f, 1.0)
        mask3 = consts.tile([128, 3], F32, tag="mask3")
        nc.vector.memset(mask3, 1.0)
        nc.vector.memset(mask3[dh:128, 1:2], 0.0)
        nc.vector.memset(mask3[0:dh, 2:3], 0.0)
        lneps_c = consts.tile([128, 1], F32, tag="lneps")
        nc.vector.memset(lneps_c, LN_EPS)
        gammaB = consts.tile([128, hdim], F32, tag="gammaB")
        nc.gpsimd.dma_start(out=gammaB, in_=bcast_row(gamma_d[:, :]))
        betaB = consts.tile([128, hdim], F32, tag="betaB")
        nc.gpsimd.dma_start(out=betaB, in_=bcast_row(beta_d[:, :]))
        bv_sb = consts.tile([hdim, 1], F32, tag="bv")
        nc.gpsimd.dma_start(out=bv_sb, in_=bv_d[:, :])
        wk_sb = consts.tile([128, 128], BF16, tag="wk")
        nc.gpsimd.dma_start(out=wk_sb, in_=wk_d[:, :])
        wmu_sb = consts.tile([128, 128], BF16, tag="wmu")
        nc.gpsimd.dma_start(out=wmu_sb, in_=wmu_d[:, :])
        wls_sb = consts.tile([128, 128], BF16, tag="wls")
        nc.gpsimd.dma_start(out=wls_sb, in_=wls_d[:, :])
        wv0_sb = consts.tile([128, 128], BF16, tag="wv0")
        nc.gpsimd.dma_start(out=wv0_sb, in_=wv0_d[:, :])
        wv1_sb = consts.tile([128, 128], BF16, tag="wv1")
        nc.gpsimd.dma_start(out=wv1_sb, in_=wv1_d[:, :])

        # ---------------- persistent big buffers ----------------
        hn_sb = big.tile([128, nblk, 128], BF16, tag="hn")        # hn, y-block tiles
        hnT_sb = big.tile([128, n], BF16, tag="hnT")              # hn^T [f, node]
        hnxT_sb = big.tile([128, xs], BF16, tag="hnxT")           # hn_x^T [f, xq]
        kT_sb = big.tile([128, xs], BF16, tag="kT")               # packed k^T
        muT_sb = big.tile([128, n], BF16, tag="muT")
        lsT_sb = big.tile([128, n], BF16, tag="lsT")
        kl_slab = big.tile([128, 2 * nacc + 2], F32, tag="klslab")
        scr = big.tile([128, chunk_blks * xs], BF16, tag="scr")   # lg/s2 scratch

        # persistent PSUM: message accumulators + denominators
        msg_ps = [msgp.tile([128, xs], F32, tag=f"msg{b}", name=f"msg{b}")
                  for b in range(heads)]
        den_ps = msgp.tile([64, xs], F32, tag="den")

        # ---------------- Phase 0: LayerNorm (+ transposes) ----------------
        with tc.tile_pool(name="lnp", bufs=3) as lnp, \
             tc.tile_pool(name="lnps", bufs=3, space="PSUM") as lnps:

            def ln_tile(src_slice, dst_bf_slice):
                x_t = lnp.tile([128, hdim], F32, tag="x")
                nc.gpsimd.dma_start(out=x_t, in_=src_slice)
                stats = lnp.tile([128, 6], F32, tag="stats")
                nc.vector.bn_stats(out=stats, in_=x_t)
                mv = lnp.tile([128, 2], F32, tag="mv")
                nc.vector.bn_aggr(out=mv, in_=stats)
                # rstd = exp(-0.5*ln(var+eps))  (stays in the ln/exp table set)
                u = lnp.tile([128, 1], F32, tag="u")
                nc.scalar.activation(out=u, in_=mv[:, 1:2], func=ACT.Ln,
                                     bias=lneps_c)
                r = lnp.tile([128, 1], F32, tag="r")
                nc.scalar.activation(out=r, in_=u, func=ACT.Exp, scale=-0.5)
                t1 = lnp.tile([128, hdim], F32, tag="t1")
                nc.vector.tensor_scalar(t1, x_t, mv[:, 0:1], r, ALU.subtract, ALU.mult)
                t2 = lnp.tile([128, hdim], F32, tag="t2")
                nc.vector.tensor_tensor(t2, t1, gammaB, ALU.mult)
                nc.vector.tensor_tensor(dst_bf_slice, t2, betaB, ALU.add)

            for blk in range(nblk):
                ln_tile(h_d[blk * 128:(blk + 1) * 128, :], hn_sb[:, blk, :])
                tp = lnps.tile([128, 128], BF16, tag="tp")
                nc.tensor.transpose(tp, hn_sb[:, blk, :], id_bf)
                nc.vector.tensor_copy(hnT_sb[:, blk * 128:(blk + 1) * 128], tp)
            for i in range(xtiles):
                hx_bf = lnp.tile([128, hdim], BF16, tag=f"hxbf{i}")
                ln_tile(hx_d[i * 128:(i + 1) * 128, :], hx_bf)
                tp = lnps.tile([128, 128], BF16, tag="tp")
                nc.tensor.transpose(tp, hx_bf, id_bf)
                nc.vector.tensor_copy(hnxT_sb[:, i * 128:(i + 1) * 128], tp)

        # ---------------- Phase 1: projections + Gram (sum mu^2) ----------
        with tc.tile_pool(name="prj", bufs=3) as prj, \
             tc.tile_pool(name="prjps", bufs=2, space="PSUM") as prjps, \
             tc.tile_pool(name="gramps", bufs=1, space="PSUM") as gramps:
            ps = prjps.tile([128, xs], F32, tag="pp")
            nc.tensor.matmul(ps, wk_sb, hnxT_sb, start=True, stop=True)
            nc.vector.tensor_copy(kT_sb, ps)
            for j in range(n // 512):
                sl = slice(j * 512, (j + 1) * 512)
                ps = prjps.tile([128, 512], F32, tag="pp")
                nc.tensor.matmul(ps, wmu_sb, hnT_sb[:, sl], start=True, stop=True)
                nc.vector.tensor_copy(muT_sb[:, sl], ps)
                ps = prjps.tile([128, 512], F32, tag="pp")
                nc.tensor.matmul(ps, wls_sb, hnT_sb[:, sl], start=True, stop=True)
                nc.vector.tensor_copy(lsT_sb[:, sl], ps)

            # Gram matrices: A = K^T K (over the x shard), B = M^T M (all nodes)
            A_ps = gramps.tile([128, 128], F32, tag="A")
            B_ps = gramps.tile([128, 128], F32, tag="B")
            for i in range(xtiles):
                ps = prjps.tile([128, 128], F32, tag="pp")
                nc.tensor.matmul(ps, hnxT_sb[:, i * 128:(i + 1) * 128], wk_sb,
                                 start=True, stop=True)
                kn = prj.tile([128, 128], BF16, tag="kn")
                nc.vector.tensor_copy(kn, ps)
                nc.tensor.matmul(A_ps, kn, kn, start=(i == 0), stop=(i == xtiles - 1))
            for j in range(nblk):
                ps = prjps.tile([128, 128], F32, tag="pp")
                nc.tensor.matmul(ps, hnT_sb[:, j * 128:(j + 1) * 128], wmu_sb,
                                 start=True, stop=True)
                mn = prj.tile([128, 128], BF16, tag="mn")
                nc.vector.tensor_copy(mn, ps)
                nc.tensor.matmul(B_ps, mn, mn, start=(j == 0), stop=(j == nblk - 1))
            A_sb = prj.tile([128, 128], F32, tag="Asb")
            nc.vector.tensor_copy(A_sb, A_ps)
            B_sb = prj.tile([128, 128], F32, tag="Bsb")
            nc.vector.tensor_copy(B_sb, B_ps)
            AB = prj.tile([128, 128], F32, tag="AB")
            nc.vector.tensor_tensor(AB, A_sb, B_sb, ALU.mult)
            nc.vector.tensor_reduce(kl_slab[:, 2 * nacc:2 * nacc + 1],
                                    AB[:, 0:dh], axis=AX.X, op=ALU.add)
            nc.vector.tensor_reduce(kl_slab[:, 2 * nacc + 1:2 * nacc + 2],
                                    AB[:, dh:128], axis=AX.X, op=ALU.add)

        # ---------------- Phase 2: main loop over key-node chunks ----------
        with tc.tile_pool(name="str", bufs=2) as strm, \
             tc.tile_pool(name="sigp", bufs=3) as sigp, \
             tc.tile_pool(name="mls", bufs=3, space="PSUM") as mls:
            for c in range(nchunk):
                y0 = c * chunk_blks * 128
                eps_t = []
                for b in range(heads):
                    et = strm.tile([128, chunk_blks, xs], BF16, tag=f"eps{b}")
                    src = eps_d[b, y0:y0 + chunk_blks * 128, :]
                    nc.sync.dma_start(
                        out=et, in_=src.rearrange("(blk p) x -> p blk x", p=128))
                    eps_t.append(et)
                dif_t = strm.tile([128, chunk_blks, xs], BF16, tag="dif")
                nc.sync.dma_start(
                    out=dif_t,
                    in_=diff_d[y0:y0 + chunk_blks * 128, :].rearrange(
                        "(blk p) x -> p blk x", p=128))

                ea_t = []
                for b in range(heads):
                    hs = slice(b * dh, (b + 1) * dh)
                    sig = sigp.tile([128, chunk_blks, xs], BF16, tag="sig")
                    # e1 = exp(ls) per block (drains score PSUM quickly)
                    for blk in range(chunk_blks):
                        yb = c * chunk_blks + blk
                        ls_ps = mls.tile([128, xs], F32, tag="ls")
                        nc.tensor.matmul(ls_ps,
                                         lsT_sb[hs, yb * 128:(yb + 1) * 128],
                                         kT_sb[hs, :], start=True, stop=True)
                        nc.scalar.activation(out=sig[:, blk, :], in_=ls_ps,
                                             func=ACT.Exp)
                    # sigma = ln(e1 + 1), chunked, in place
                    nc.scalar.activation(out=sig, in_=sig, func=ACT.Ln, bias=1.0)
                    # KL terms: sum ln(sigma), sum sigma^2
                    nc.scalar.activation(out=scr, in_=sig, func=ACT.Ln,
                                         accum_out=kl_slab[:, c * heads + b:
                                                           c * heads + b + 1])
                    nc.vector.scalar_tensor_tensor(
                        out=scr, in0=sig, scalar=1.0, in1=sig,
                        op0=ALU.bypass, op1=ALU.mult,
                        accum_out=kl_slab[:, nacc + c * heads + b:
                                          nacc + c * heads + b + 1])
                    # p = sigma * eps
                    p_t = sigp.tile([128, chunk_blks, xs], BF16, tag="p")
                    nc.vector.tensor_tensor(p_t, sig, eps_t[b], ALU.mult)
                    # t = p + mu (PE adds the mu scores on top of I@p), e = exp(t)
                    ea = sigp.tile([128, chunk_blks, xs], BF16, tag=f"ea{b}")
                    for blk in range(chunk_blks):
                        yb = c * chunk_blks + blk
                        t_ps = mls.tile([128, xs], F32, tag="t", bufs=2)
                        nc.tensor.matmul(t_ps, id_bf, p_t[:, blk, :],
                                         start=True, stop=False)
                        nc.tensor.matmul(t_ps,
                                         muT_sb[hs, yb * 128:(yb + 1) * 128],
                                         kT_sb[hs, :], start=False, stop=True)
                        nc.scalar.activation(out=ea[:, blk, :], in_=t_ps,
                                             func=ACT.Exp)
                    # a = e * diffusion (in place)
                    nc.vector.tensor_tensor(ea, ea, dif_t, ALU.mult)
                    ea_t.append(ea)

                # message passing + denominators
                for blk in range(chunk_blks):
                    yb = c * chunk_blks + blk
                    st = (yb == 0)
                    sp = (yb == nblk - 1)
                    for b in range(heads):
                        nc.tensor.matmul(msg_ps[b], hn_sb[:, yb, :],
                                         ea_t[b][:, blk, :], start=st, stop=sp)
                    nc.tensor.matmul(den_ps[0:1, :], ones_bf[:, 0:1],
                                     ea_t[0][:, blk, :], start=st, stop=sp,
                                     skip_group_check=True)
                    nc.tensor.matmul(den_ps[32:33, :], ones_bf[:, 0:1],
                                     ea_t[1][:, blk, :], start=st, stop=sp,
                                     skip_group_check=True)

        # ---------------- Phase 3: normalize, output proj, ELU, KL --------
        with tc.tile_pool(name="tl", bufs=1) as tl, \
             tc.tile_pool(name="tlps", bufs=1, space="PSUM") as tlps:
            msc = []
            for b in range(heads):
                d_sb = tl.tile([1, xs], F32, tag=f"d{b}")
                nc.vector.tensor_copy(d_sb, den_ps[32 * b:32 * b + 1, :])
                nc.vector.tensor_scalar_max(d_sb, d_sb, L1_EPS)
                nc.vector.reciprocal(d_sb, d_sb)
                rb_ps = tlps.tile([128, xs], F32, tag=f"rb{b}")
                nc.tensor.matmul(rb_ps, ones1f[0:1, :], d_sb, start=True, stop=True)
                rb_sb = tl.tile([128, xs], F32, tag=f"rbs{b}")
                nc.vector.tensor_copy(rb_sb, rb_ps)
                m_sb = tl.tile([128, xs], BF16, tag=f"msc{b}")
                nc.vector.tensor_tensor(m_sb, msg_ps[b], rb_sb, ALU.mult)
                msc.append(m_sb)
            o_ps = tlps.tile([128, xs], F32, tag="o")
            nc.tensor.matmul(o_ps, wv0_sb, msc[0], start=True, stop=False)
            nc.tensor.matmul(o_ps, wv1_sb, msc[1], start=False, stop=True)
            # elu(x) = max(x, min(0, exp(x) - 1)), with bias b_v folded in
            xb = tl.tile([128, xs], F32, tag="xb")
            nc.vector.tensor_scalar(xb, o_ps, bv_sb, None, ALU.add)
            ex = tl.tile([128, xs], F32, tag="ex")
            nc.scalar.activation(out=ex, in_=o_ps, func=ACT.Exp, bias=bv_sb)
            mn = tl.tile([128, xs], F32, tag="mn")
            nc.vector.tensor_scalar(mn, ex, -1.0, 0.0, ALU.add, ALU.min)
            ot = tl.tile([128, xs], F32, tag="ot")
            nc.vector.tensor_tensor(ot, xb, mn, ALU.max)
            nc.gpsimd.dma_start(out=ot_d[:, :], in_=ot)

            # KL assembly
            kw = 2 * nacc + 2
            klps = tlps.tile([1, kw], F32, tag="klps")
            nc.tensor.matmul(klps[0:1, 0:2 * nacc], mask3[:, 0:1],
                             kl_slab[:, 0:2 * nacc], start=True, stop=True)
            nc.tensor.matmul(klps[0:1, 2 * nacc:2 * nacc + 1], mask3[:, 1:2],
                             kl_slab[:, 2 * nacc:2 * nacc + 1],
                             start=True, stop=True)
            nc.tensor.matmul(klps[0:1, 2 * nacc + 1:kw], mask3[:, 2:3],
                             kl_slab[:, 2 * nacc + 1:kw], start=True, stop=True)
            ksb = tl.tile([1, kw], F32, tag="ksb")
            nc.vector.tensor_copy(ksb, klps)
            r_lg = tl.tile([1, 1], F32, tag="rlg")
            nc.vector.tensor_reduce(r_lg, ksb[0:1, 0:nacc], axis=AX.X, op=ALU.add)
            r_s2 = tl.tile([1, 1], F32, tag="rs2")
            nc.vector.tensor_reduce(r_s2, ksb[0:1, nacc:2 * nacc], axis=AX.X,
                                    op=ALU.add)
            mu2 = tl.tile([1, 1], F32, tag="mu2")
            nc.vector.tensor_tensor(mu2, ksb[0:1, 2 * nacc:2 * nacc + 1],
                                    ksb[0:1, 2 * nacc + 1:kw], ALU.add)
            u1 = tl.tile([1, 1], F32, tag="u1")
            nc.vector.tensor_scalar(u1, r_lg, -1.0, None, ALU.mult)
            u2 = tl.tile([1, 1], F32, tag="u2")
            nc.vector.tensor_scalar(u2, r_s2, 0.5, None, ALU.mult)
            nc.vector.tensor_tensor(u1, u1, u2, ALU.add)
            nc.vector.tensor_scalar(u2, mu2, 0.5, None, ALU.mult)
            nc.vector.tensor_tensor(u1, u1, u2, ALU.add)
            cnt = float(xs) * n * heads
            klv = tl.tile([1, 1], F32, tag="klv")
            nc.vector.tensor_scalar(klv, u1, 1.0 / (float(n) * n),
                                    -0.5 * cnt / (float(n) * n), ALU.mult, ALU.add)
            nc.gpsimd.dma_start(out=kl_d[:, :], in_=klv)

    # Pin every activation to the one table set containing both Exp and Ln,
    # otherwise the per-instruction set chooser ping-pongs between
    # exp_and_others and natural_log (~1.3us ACT_TABLE_LOAD each, x84).
    _orig_tables = bacc.get_activation_tables
    _KEEP = "natural_log_exp_and_others"

    def _pinned_tables(arch):
        tabs = _orig_tables(arch)
        return {name: (fns if name == _KEEP else set())
                for name, fns in tabs.items()}

    bacc.get_activation_tables = _pinned_tables
    try:
        nc.compile()
    finally:
        bacc.get_activation_tables = _orig_tables
    return nc


# ------------------------------------------------------------------------
# Host side
# ------------------------------------------------------------------------

def prep_inputs(inputs, n=4096, xs=512, hdim=128, heads=2):
    """Shard + lay out the full-size numpy inputs into per-core in_maps."""
    h = np.asarray(inputs["h"], np.float32)
    diffusion = np.asarray(inputs["diffusion"], np.float32)
    eps = np.asarray(inputs["eps"], np.float32)
    w_k = np.asarray(inputs["w_k"], np.float32)
    w_mu = np.asarray(inputs["w_mu"], np.float32)
    w_ls = np.asarray(inputs["w_ls"], np.float32)
    w_v = np.asarray(inputs["w_v"], np.float32)
    b_v = np.asarray(inputs["b_v"], np.float32)
    gamma = np.asarray(inputs["gamma"], np.float32)
    beta = np.asarray(inputs["beta"], np.float32)

    scale = hdim ** -0.5

    def pack(w):
        return np.concatenate([w[b::heads] for b in range(heads)], 0)

    wk2T = np.ascontiguousarray((pack(w_k) * scale).T).astype(NP_BF16)
    wmu2T = np.ascontiguousarray(pack(w_mu).T).astype(NP_BF16)
    wls2T = np.ascontiguousarray(pack(w_ls).T).astype(NP_BF16)
    wv0T = np.ascontiguousarray(w_v[:, 0:hdim].T).astype(NP_BF16)
    wv1T = np.ascontiguousarray(w_v[:, hdim:2 * hdim].T).astype(NP_BF16)
    shared = {
        "h": np.ascontiguousarray(h),
        "gamma": np.ascontiguousarray(gamma.reshape(1, hdim)),
        "beta": np.ascontiguousarray(beta.reshape(1, hdim)),
        "b_v": np.ascontiguousarray(b_v.reshape(hdim, 1)),
        "wk2T": wk2T, "wmu2T": wmu2T, "wls2T": wls2T,
        "wv0T": wv0T, "wv1T": wv1T,
    }
    in_maps = []
    e0 = eps[0]                      # [N, N, heads]  (x, y, b)
    d0 = diffusion[:, :, 0]          # [N, N]         (x, y)
    for i in range(N_CORES):
        x0, x1 = i * xs, (i + 1) * xs
        m = dict(shared)
        m["h_x"] = np.ascontiguousarray(h[x0:x1])
        m["epsT"] = np.ascontiguousarray(
            e0[x0:x1].transpose(2, 1, 0)).astype(NP_BF16)
        m["diffT"] = np.ascontiguousarray(d0[x0:x1].T).astype(NP_BF16)
        in_maps.append(m)
    return in_maps


_NC_CACHE = {}


def _get_nc():
    if "nc" not in _NC_CACHE:
        _NC_CACHE["nc"] = build()
    return _NC_CACHE["nc"]


def _install_ntff_hook():
    """Shim antenv.axon_hooks (missing in this image) so trace=True works."""
    import types
    import antenv  # noqa: F401
    if "antenv.axon_hooks" in sys.modules:
        return
    mod = types.ModuleType("antenv.axon_hooks")
    mod._hook = None
    mod.set_axon_ntff_profile_hook = lambda h: setattr(mod, "_hook", h)
    mod.get_axon_ntff_profile_hook = lambda: mod._hook
    sys.modules["antenv.axon_hooks"] = mod
    setattr(antenv, "axon_hooks", mod)
    if "/root/.axon_site" not in sys.path:
        sys.path.insert(0, "/root/.axon_site")
    from trn_agent_boot.trn_boot import _ntff_profile_via_ctypes
    mod._hook = _ntff_profile_via_ctypes("/opt/axon/libaxon_pjrt.so")
    import concourse.bass_utils as bu
    bu.upload_artifacts = lambda d: d


def _run(inputs, trace=False):
    from concourse.bass_utils import run_bass_kernel_spmd
    if trace:
        _install_ntff_hook()
    nc = _get_nc()
    in_maps = prep_inputs(inputs)
    res = run_bass_kernel_spmd(nc, in_maps, list(range(N_CORES)), trace=trace)
    out = np.empty((4096, 128), np.float32)
    kl = np.float32(0.0)
    for i in range(N_CORES):
        out[i * 512:(i + 1) * 512, :] = res.results[i]["o_t"].T
        kl += np.float32(res.results[i]["kl"][0, 0])
    return (out, kl), res


def kernel(**inputs):
    (out, kl), _ = _run(inputs, trace=False)
    return out, kl


def kernel_timed(**inputs):
    (out, kl), res = _run(inputs, trace=True)
    return (out, kl), res.exec_time_ns
